# revision 66
# baseline (speedup 1.0000x reference)
"""Trainium2 Bass kernel for GQA attention with RoPE (dense_transformer).

Model: B=2, T=2048, C=2048, H=16 query heads, KV=4 kv heads, D=128, causal.

Sharding (v4): 8 cores = batch(2) x kv-group(4) tensor parallel for the
projections and attention. For the output projection the parallelism is
switched to sequence via a per-group AllGather of the small attention-output
factors (0.5MB in, ~4x cheaper per byte than ReduceScatter, which is
reduce-compute-bound on the CC cores): every core gathers all 16 heads'
attnT for the group, selects its own quarter of the rows with a host-supplied
one-hot mask (keeps the program rank-independent for SPMD), and computes the
full output projection for those rows locally - the result needs no further
collective.

Layouts: all operands arrive pre-transposed/permuted/cast bf16 from the host
(contraction dim on partitions); scores are computed transposed ([tk, tq]) so
exp'd probabilities feed PV directly from the activation engine's exp
eviction; softmax denominators come from a ones-vector matmul; normalization
broadcasts the raw sums with a K=1 matmul, reciprocals on a full [128, tq]
tile, and multiplies - deferred one head-pair so none of it blocks the tensor
engine.
"""

import os

os.environ.setdefault("MYCRO_LOCAL_CACHE", "1")

import numpy as np

B, T, C = 2, 2048, 2048
H, KV, D = 16, 4, 128
HL = H // KV          # 4 local query heads per core
NCORES = 8
P = 128
SCALE = 1.0 / float(np.sqrt(D))

NCC = C // P          # 16 c-chunks
NTC = T // 512        # 4 t-chunks of 512
NG = T // 512         # 4 q-block groups (512 queries each)
TQ = 512              # queries per attention group
NEG = -1e10


def _emit(nc, tile, mybir, ExitStack):
    from concourse.masks import make_identity

    f32 = mybir.dt.float32
    bf16 = mybir.dt.bfloat16
    Exp = mybir.ActivationFunctionType.Exp
    Copy = mybir.ActivationFunctionType.Copy
    add = mybir.AluOpType.add
    mult = mybir.AluOpType.mult

    xT = nc.dram_tensor("xT", [NTC, C, 512], bf16, kind="ExternalInput")
    wqkvT = nc.dram_tensor("wqkvT", [C, HL * D + 2 * D], bf16,
                           kind="ExternalInput")
    woT = nc.dram_tensor("woT", [H * D, C], bf16, kind="ExternalInput")
    cost = nc.dram_tensor("cost", [P, T], bf16, kind="ExternalInput")
    sint = nc.dram_tensor("sint", [P, T], bf16, kind="ExternalInput")
    qsel = nc.dram_tensor("qsel", [P, 4], f32, kind="ExternalInput")
    out = nc.dram_tensor("out", [T // 4, C], bf16, kind="ExternalOutput")

    te, ve, sc, gp, sy = nc.tensor, nc.vector, nc.scalar, nc.gpsimd, nc.sync

    with tile.TileContext(nc) as tc, ExitStack() as ctx:
        consts = ctx.enter_context(tc.tile_pool(name="consts", bufs=1))
        persist = ctx.enter_context(tc.tile_pool(name="persist", bufs=1))
        dram = ctx.enter_context(tc.tile_pool(name="dram", bufs=1, space="DRAM"))

        ident = consts.tile([P, P], bf16, tag="ident")
        make_identity(nc, ident[:])
        # scoresT layout [tk, tq]: keep where tq >= tk, else -1e10.
        triT = consts.tile([P, P], f32, tag="triT")
        gp.memset(triT[:], 0.0)
        gp.affine_select(
            out=triT[:], in_=triT[:], compare_op=mybir.AluOpType.is_ge,
            fill=NEG, base=0, pattern=[[1, P]], channel_multiplier=-1,
        )
        ones = consts.tile([P, 1], bf16, tag="ones")
        gp.memset(ones[:], 1.0)
        onesc = consts.tile([P, P], bf16, tag="onesc")
        gp.memset(onesc[:], 1.0)
        qsel_sb = consts.tile([P, 4], f32, tag="qsel")
        gp.dma_start(qsel_sb[:], qsel.ap()[:, :])

        woTs = [persist.tile([P, C], bf16, tag=f"woT{ch}", name=f"woT{ch}")
                for ch in range(NCC)]
        qrT = [persist.tile([P, T], bf16, tag=f"qrT{h}", name=f"qrT{h}")
               for h in range(HL)]
        krT = persist.tile([P, T], bf16, tag="krT")
        vnat = persist.tile([P, T], bf16, tag="vnat")

        # one gather per head-pair: [2 heads x 128 dv, TQ] in, x4 cores out
        ag_in = [[dram.tile([2 * P, TQ], bf16, tag=f"agin{g}_{p}",
                            name=f"agin{g}_{p}") for p in range(2)]
                 for g in range(NG)]
        ag_out = [[dram.tile([8 * P, TQ], bf16, tag=f"agout{g}_{p}",
                             name=f"agout{g}_{p}") for p in range(2)]
                  for g in range(NG)]

        # ---- projection phase (pools freed afterwards) ------------------------
        with tc.tile_pool(name="projW", bufs=1) as projw, \
             tc.tile_pool(name="projX", bufs=2) as projx, \
             tc.tile_pool(name="stB", bufs=4) as sbB, \
             tc.tile_pool(name="ropetmp", bufs=3) as sbR, \
             tc.tile_pool(name="psB", bufs=3, space="PSUM") as psB, \
             tc.tile_pool(name="psV", bufs=2, space="PSUM") as psV:

            ck = projw.tile([P, T], bf16, tag="ck")
            sk = projw.tile([P, T], bf16, tag="sk")
            wq_t = [projw.tile([P, HL * P + 2 * P], bf16, tag=f"wq{cc}",
                               name=f"wq{cc}") for cc in range(NCC)]

            def load_xchunk(tc4):
                tiles = []
                for cc in range(NCC):
                    t_ = projx.tile([P, 512], bf16, tag=f"xc{cc}",
                                    name=f"xc{cc}")
                    (sy, sc)[cc % 2].dma_start(
                        t_[:], xT.ap()[tc4, cc * P:(cc + 1) * P, :])
                    tiles.append(t_)
                return tiles

            # interleave qkv weights with x chunk 0 so the first chains
            # stream as tiles land
            xtiles = []
            for cc in range(NCC):
                (sy, sc, gp)[cc % 3].dma_start(
                    wq_t[cc][:], wqkvT.ap()[cc * P:(cc + 1) * P, :])
                t_ = projx.tile([P, 512], bf16, tag=f"xc{cc}", name=f"xc{cc}")
                (sc, gp, sy)[cc % 3].dma_start(
                    t_[:], xT.ap()[0, cc * P:(cc + 1) * P, :])
                xtiles.append(t_)
            gp.dma_start(ck[:], cost.ap()[:, :])
            sy.dma_start(sk[:], sint.ap()[:, :])

            def rope(dst, qs, cos_t, sin_t, sl):
                q1lo = sbR.tile([64, 512], bf16, tag="q1lo", name="q1lo")
                sy.dma_start(q1lo[:], qs[64:P, :])
                q0hi = sbR.tile([P, 512], bf16, tag="q0hi", name="q0hi")
                sy.dma_start(q0hi[64:P, :], qs[0:64, :])
                ta = sbR.tile([64, 512], bf16, tag="rta", name="rta")
                tb2 = sbR.tile([64, 512], bf16, tag="rtb", name="rtb")
                ve.tensor_mul(ta[:], qs[0:64, :], cos_t[0:64, sl])
                ve.tensor_mul(tb2[:], q1lo[:], sin_t[0:64, sl])
                ve.tensor_sub(dst[0:64, sl], ta[:], tb2[:])
                tc2 = sbR.tile([P, 512], bf16, tag="rtc", name="rtc")
                td = sbR.tile([P, 512], bf16, tag="rtd", name="rtd")
                ve.tensor_mul(tc2[64:P, :], q0hi[64:P, :], sin_t[64:P, sl])
                ve.tensor_mul(td[64:P, :], qs[64:P, :], cos_t[64:P, sl])
                ve.tensor_add(dst[64:P, sl], tc2[64:P, :], td[64:P, :])

            for tc4 in range(NTC):
                nxt = load_xchunk(tc4 + 1) if tc4 + 1 < NTC else None
                sl = slice(tc4 * 512, (tc4 + 1) * 512)
                # 4 q heads
                for h in range(HL):
                    ps = psB.tile([P, 512], f32, tag="pqkv", name="pqkv")
                    for cc in range(NCC):
                        te.matmul(ps[:], wq_t[cc][:, h * P:(h + 1) * P],
                                  xtiles[cc][:],
                                  start=(cc == 0), stop=(cc == NCC - 1))
                    qs = sbB.tile([P, 512], bf16, tag="qkev", name="qkev")
                    sc.activation(qs[:], ps[:], Copy)
                    rope(qrT[h], qs, ck, sk, sl)
                # k
                ps = psB.tile([P, 512], f32, tag="pqkv", name="pqkv")
                for cc in range(NCC):
                    te.matmul(ps[:], wq_t[cc][:, HL * P:HL * P + P],
                              xtiles[cc][:],
                              start=(cc == 0), stop=(cc == NCC - 1))
                qs = sbB.tile([P, 512], bf16, tag="qkev", name="qkev")
                sc.activation(qs[:], ps[:], Copy)
                rope(krT, qs, ck, sk, sl)
                # v (natural layout via an extra transpose)
                ps = psB.tile([P, 512], f32, tag="pqkv", name="pqkv")
                for cc in range(NCC):
                    te.matmul(ps[:], wq_t[cc][:, HL * P + P:],
                              xtiles[cc][:],
                              start=(cc == 0), stop=(cc == NCC - 1))
                vt = sbB.tile([P, 512], bf16, tag="qkev", name="qkev")
                sc.activation(vt[:], ps[:], Copy)
                pv = psV.tile([P, 512], bf16, tag="pvtr", name="pvtr")
                for i in range(4):
                    te.transpose(pv[:, i * P:(i + 1) * P],
                                 vt[:, i * P:(i + 1) * P], ident[:])
                sc.activation(vnat[:, sl], pv[:], Copy)
                xtiles = nxt

        # ---- attention + gather + output projection ---------------------------
        with tc.tile_pool(name="attnP", bufs=1) as sbAT, \
             tc.tile_pool(name="probsP", bufs=8) as sbP, \
             tc.tile_pool(name="stC", bufs=3) as sbC, \
             tc.tile_pool(name="stCbc", bufs=4) as sbBC, \
             tc.tile_pool(name="stAg", bufs=1) as sbAG, \
             tc.tile_pool(name="stSel", bufs=2) as sbSel, \
             tc.tile_pool(name="stD", bufs=3) as sbD, \
             tc.tile_pool(name="psT", bufs=2, space="PSUM") as psT, \
             tc.tile_pool(name="psAttn", bufs=2, space="PSUM") as psAt, \
             tc.tile_pool(name="psSums", bufs=1, space="PSUM") as psSm, \
             tc.tile_pool(name="psY", bufs=2, space="PSUM") as psY, \
             tc.tile_pool(name="psBC", bufs=1, space="PSUM") as psBC:

            # full wo arrives during early attention
            for ch in range(NCC):
                (sy, sc, gp)[ch % 3].dma_start(
                    woTs[ch][:], woT.ap()[ch * P:(ch + 1) * P, :])

            attnT = [[sbAT.tile([P, TQ], bf16, tag=f"attnT{h}_{g}",
                                 name=f"attnT{h}_{g}") for g in range(NG)]
                     for h in range(HL)]

            def emit_scores(gq, kb, hs):
                """scoresT + exp for one k-block, two heads; returns probs."""
                j = kb - 4 * gq
                w0 = max(j, 0) * P
                probs = []
                for h in hs:
                    st = psT.tile([P, TQ], f32, tag="pscore", name="pscore")
                    te.matmul(
                        st[:, w0:TQ],
                        krT[:, kb * P:(kb + 1) * P],
                        qrT[h][:, gq * TQ + w0:(gq + 1) * TQ],
                        start=True, stop=True,
                    )
                    if j >= 0:
                        ve.tensor_tensor(
                            st[:, w0:w0 + P], st[:, w0:w0 + P], triT[:], add)
                    pb = sbP.tile([P, TQ], bf16, tag="probs", name="probs")
                    sc.activation(pb[:, w0:TQ], st[:, w0:TQ], Exp)
                    probs.append(pb)
                return probs, w0

            def emit_accum(gq, kb, kbmax, w0, probs, prev_probs, pa, psums):
                full = kb < 4 * gq
                if not full:
                    # diagonal region: per-block denominator accumulation
                    for i in range(2):
                        te.matmul(
                            psums[64 * i:64 * i + 1, w0:TQ], ones[:],
                            probs[i][:, w0:TQ],
                            start=(kb == 0), stop=(kb == kbmax - 1),
                        )
                elif kb % 2 == 1:
                    # below the diagonal: pre-sum the pair of blocks on the
                    # vector engine, halving the ones-matmul work
                    for i in range(2):
                        padd = sbP.tile([P, TQ], bf16, tag="padd",
                                        name="padd")
                        ve.tensor_add(padd[:], probs[i][:], prev_probs[i][:])
                        te.matmul(
                            psums[64 * i:64 * i + 1, :], ones[:], padd[:],
                            start=(kb == 1), stop=False,
                        )
                for i in range(2):
                    te.matmul(
                        pa[i][:, w0:TQ], vnat[:, kb * P:(kb + 1) * P],
                        probs[i][:, w0:TQ],
                        start=(kb == 0), stop=(kb == kbmax - 1),
                    )

            def emit_norm_head(ent, i):
                """broadcast sums -> reciprocal -> scale attnT -> stage."""
                sums_sb, gq, hs = ent
                h = hs[i]
                pbc = psBC.tile([P, TQ], f32, tag="pbc", name="pbc")
                te.matmul(pbc[:], onesc[64 * i:64 * i + 1, :],
                          sums_sb[64 * i:64 * i + 1, :],
                          start=True, stop=True)
                recip = sbBC.tile([P, TQ], f32, tag="rbc", name="rbc")
                ve.reciprocal_approx_fast(recip[:], pbc[:])
                ve.tensor_mul(attnT[h][gq][:], attnT[h][gq][:], recip[:])
                # stage this head's factor for the gather
                sy.dma_start(
                    ag_in[gq][h // 2][(h % 2) * P:(h % 2 + 1) * P, :],
                    attnT[h][gq][:])

            # outproj accumulation order: pair-0-sourced chunks first, so the
            # chain can start as soon as the early gather lands
            CH_ORDER = sorted(range(NCC), key=lambda ch: ((ch % HL) // 2, ch))

            def emit_ag(gq, p):
                gp.collective_compute(
                    "AllGather", mybir.AluOpType.bypass,
                    replica_groups=[[0, 1, 2, 3], [4, 5, 6, 7]],
                    ins=[ag_in[gq][p][:].rearrange("a b -> (a b)").opt()],
                    outs=[ag_out[gq][p][:].rearrange("a b -> (a b)").opt()],
                )

            def emit_gather_load_half(gq, p, asel):
                """loadback + own-quarter select for one head-pair's gather
                (DMA + DVE only); both heads of the pair load into one
                [P, 2, TQ] tile and select together."""
                for g in range(4):
                    src = ag_out[gq][p]
                    row0 = g * 2 * P
                    agt = sbAG.tile([P, 2, TQ], bf16, tag=f"agt{g}_{p}",
                                    name=f"agt{g}_{p}")
                    gp.dma_start(
                        agt[:],
                        src[row0:row0 + 2 * P, :]
                        .rearrange("(h r) t -> r h t", h=2))
                    sel = sbSel.tile([P, 2, P], bf16, tag=f"sel{g}_{p}",
                                     name=f"sel{g}_{p}")
                    ve.tensor_scalar_mul(sel[:], agt[:, :, 0:P],
                                         qsel_sb[:, 0:1])
                    for q in range(1, 4):
                        ve.scalar_tensor_tensor(
                            sel[:], agt[:, :, q * P:(q + 1) * P],
                            qsel_sb[:, q:q + 1], sel[:], mult, add)
                    for i in range(2):
                        asel[g * HL + p * 2 + i] = sel[:, i, :]

            def emit_gather_load(gq):
                asel = {}
                emit_gather_load_half(gq, 0, asel)
                emit_gather_load_half(gq, 1, asel)
                return asel

            def emit_outproj_chunk(gq, asel, cc4, ysb):
                py = psY.tile([P, 512], f32, tag="py", name="py")
                for n, ch in enumerate(CH_ORDER):
                    te.matmul(
                        py[:], asel[ch],
                        woTs[ch][:, cc4 * 512:(cc4 + 1) * 512],
                        start=(n == 0), stop=(n == NCC - 1),
                    )
                if cc4 % 2 == 0:
                    ve.tensor_copy(ysb[:, cc4 * 512:(cc4 + 1) * 512], py[:])
                else:
                    sc.activation(ysb[:, cc4 * 512:(cc4 + 1) * 512], py[:],
                                  Copy)
                if cc4 == 3:
                    sy.dma_start(out.ap()[gq * P:(gq + 1) * P, :], ysb[:])

            proj_q = None       # gq whose gather is in flight
            proj_sel = None     # its loaded+selected factors

            for gq in range(NG):
                kbmax = 4 * (gq + 1)
                for hp in range(HL // 2):
                    hs = (2 * hp, 2 * hp + 1)
                    last_pair = hp == HL // 2 - 1
                    pa = [psAt.tile([P, TQ], f32, tag="pattn", name="pattn")
                          for _ in hs]
                    psums = psSm.tile([P, TQ], f32, tag="psums", name="psums")
                    if last_pair and proj_q is not None:
                        ysb_prev = sbD.tile([P, C], bf16, tag="ysb",
                                            name="ysb")
                    # software-pipelined: scores(kb+1) issue before accum(kb)
                    prev = None
                    probs_hist = {}
                    for kb in range(kbmax):
                        cur = (kb, *emit_scores(gq, kb, hs))
                        probs_hist[kb] = cur[1]
                        if last_pair and kb in (1, 2, 3, 4) \
                                and proj_q is not None:
                            # project the previous group's gathered factors,
                            # one chunk per slot to keep the pipelines fed
                            emit_outproj_chunk(proj_q, proj_sel, kb - 1,
                                               ysb_prev)
                            if kb == 4:
                                proj_q = proj_sel = None
                        if gq == NG - 1 and last_pair and kb == 12:
                            # last group: its first pair's gather landed
                            # long ago - load it so the drain can project
                            # that half while the final gather is in flight
                            drain_sel = {}
                            emit_gather_load_half(gq, 0, drain_sel)
                        if prev is not None:
                            pkb, pprobs, pw0 = prev
                            emit_accum(gq, pkb, kbmax, pw0, pprobs,
                                       probs_hist.get(pkb - 1), pa, psums)
                        prev = cur
                    pkb, pprobs, pw0 = prev
                    emit_accum(gq, pkb, kbmax, pw0, pprobs,
                               probs_hist.get(pkb - 1), pa, psums)

                    # evict sums + attention accumulators, normalize and
                    # stage immediately so the gathers launch early
                    sums_sb = sbC.tile([P, TQ], bf16, tag="sums_sb",
                                       name="sums_sb")
                    sc.activation(sums_sb[0:1, :], psums[0:1, :], Copy)
                    sc.activation(sums_sb[64:65, :], psums[64:65, :], Copy)
                    for i, h in enumerate(hs):
                        if i % 2 == 0:
                            sc.activation(attnT[h][gq][:], pa[i][:], Copy)
                        else:
                            ve.tensor_copy(attnT[h][gq][:], pa[i][:])
                    emit_norm_head((sums_sb, gq, hs), 0)
                    emit_norm_head((sums_sb, gq, hs), 1)
                    emit_ag(gq, hp)
                    if hp == 0 and proj_q is not None:
                        # previous group's gathers have landed by now: queue
                        # the loadback + select (no tensor-engine dependency)
                        proj_sel = emit_gather_load(proj_q)

                proj_q = gq

            # drain the last group: project the early-gathered half while
            # the final gather is still in flight, then accumulate the rest
            CH0 = [ch for ch in CH_ORDER if (ch % HL) // 2 == 0]
            CH1 = [ch for ch in CH_ORDER if (ch % HL) // 2 == 1]
            ysb0 = sbD.tile([P, C], bf16, tag="ysb0", name="ysb0")
            ysb_last = sbD.tile([P, C], bf16, tag="ysb", name="ysb")
            for cc4 in range(4):
                py = psY.tile([P, 512], f32, tag="py", name="py")
                csl = slice(cc4 * 512, (cc4 + 1) * 512)
                for n, ch in enumerate(CH0):
                    te.matmul(py[:], drain_sel[ch], woTs[ch][:, csl],
                              start=(n == 0), stop=(n == len(CH0) - 1))
                if cc4 % 2 == 0:
                    ve.tensor_copy(ysb0[:, csl], py[:])
                else:
                    sc.activation(ysb0[:, csl], py[:], Copy)
            emit_gather_load_half(proj_q, 1, drain_sel)
            for cc4 in range(4):
                py = psY.tile([P, 512], f32, tag="py", name="py")
                csl = slice(cc4 * 512, (cc4 + 1) * 512)
                for n, ch in enumerate(CH1):
                    te.matmul(py[:], drain_sel[ch], woTs[ch][:, csl],
                              start=(n == 0), stop=(n == len(CH1) - 1))
                ve.tensor_add(ysb_last[:, csl], py[:], ysb0[:, csl])
            sy.dma_start(out.ap()[proj_q * P:(proj_q + 1) * P, :],
                         ysb_last[:])

    return nc


_PROGRAM = None


def _get_program():
    global _PROGRAM
    if _PROGRAM is None:
        from contextlib import ExitStack
        import concourse.tile as tile
        from concourse import bacc, mybir

        nc = bacc.Bacc("TRN2", target_bir_lowering=False, debug=False,
                       num_devices=NCORES)
        _emit(nc, tile, mybir, ExitStack)
        nc.compile()
        _PROGRAM = nc
    return _PROGRAM


def make_in_maps(x, wq, wk, wv, wo, freqs_cos, freqs_sin):
    """Host-side shard + relayout: transpose to contraction-major, apply the
    rope even/odd row permutation to wq/wk, fold the 1/sqrt(D) score scale
    into the q cos/sin tables, and cast everything to bf16."""
    import ml_dtypes

    bf = ml_dtypes.bfloat16

    def c(a):
        return np.ascontiguousarray(a.astype(bf))

    x = np.asarray(x, np.float32)
    wq = np.asarray(wq, np.float32)
    wk = np.asarray(wk, np.float32)
    wv = np.asarray(wv, np.float32)
    wo = np.asarray(wo, np.float32)
    fc = np.asarray(freqs_cos, np.float32)
    fs = np.asarray(freqs_sin, np.float32)

    # even/odd rope permutation within each 128-row head block
    perm = np.concatenate([np.arange(0, D, 2), np.arange(1, D, 2)])
    woT_b = c(wo.T)  # [H*D, C], shared by all cores
    c64 = fc.T  # [64, T]
    s64 = fs.T
    cost = c(np.concatenate([c64, c64]))
    sint = c(np.concatenate([s64, s64]))

    in_maps = []
    for core in range(NCORES):
        b, g = core // 4, core % 4
        wqg = wq[g * HL * D:(g + 1) * HL * D] * SCALE  # [512, C]
        wqg = wqg.reshape(HL, D, C)[:, perm].reshape(HL * D, C)
        wkg = wk[g * D:(g + 1) * D][perm]              # [128, C]
        wvg = wv[g * D:(g + 1) * D]
        wqkv = np.concatenate([wqg, wkg, wvg], axis=0)  # [768, C]
        qsel = np.zeros((P, 4), np.float32)
        qsel[:, g] = 1.0
        in_maps.append({
            "xT": c(x[b].T.reshape(C, NTC, 512).transpose(1, 0, 2)),
            "wqkvT": c(wqkv.T),
            "woT": woT_b,
            "cost": cost, "sint": sint,
            "qsel": qsel,
        })
    return in_maps


def kernel(x, wq, wk, wv, wo, freqs_cos, freqs_sin, mask=None):
    from concourse.bass_utils import run_bass_kernel_spmd

    nc = _get_program()
    in_maps = make_in_maps(x, wq, wk, wv, wo, freqs_cos, freqs_sin)
    res = run_bass_kernel_spmd(nc, in_maps, core_ids=list(range(NCORES)))
    outp = np.empty((B, T, C), np.float32)
    for b in range(B):
        for r in range(4):
            piece = np.asarray(res.results[4 * b + r]["out"],
                               dtype=np.float32)  # [512, C] = 4 x [128, C]
            for gq in range(NG):
                dst = gq * 512 + r * P
                outp[b, dst:dst + P] = piece[gq * P:(gq + 1) * P]
    return outp
